# revision 2
# baseline (speedup 1.0000x reference)
"""Trainium2 Bass kernel for NeuralGraphOutput (gnn_message_passing).

Math (per sample b):
    out[b, :] = sum_a mask[b,a] * relu(cat(atoms[b,a,:], sum_d bonds[b,a,d,:]) @ W + bias)
    mask[b,a] = any(edges[b,a,:] != -1)

Strategy: pure data parallel over 8 NeuronCores (256 samples each).
Per core, rows = flattened (sample, atom) pairs, processed in super-tiles
of 512 rows = 4 sub-tiles of 128 rows:
  - DMA atoms -> atile, bonds -> bonds_sb (rows on partitions)
  - GpSimd folds bonds D=8->4 (bondsum2), VectorE reduces 4->1 (bondsum)
  - TensorE transposes atile [128,64] and bondsum [128,16] per sub-tile
    into PSUM feature-major columns
  - VectorE drains PSUM -> catT (rounding to fp32r); catT has preset
    zero rows 80:96 and a ones row 96 (bias fold), so K=97
  - main matmul (fp32r): psum_fp[128,256] = catT.T @ W_aug
  - ScalarE relu PSUM -> SBUF as fp16
  - reduction matmul (fp16): psum_out[16,256] += mask16[:,t,:].T @ relu
    (mask-weighted per-sample atom sum; masks precomputed on device from
     edges, laid out as one-hot columns per sub-tile)
  - after 16 samples accumulate, drain psum_out -> SBUF -> DRAM.

Instruction sync-wait budget is 2 per instruction; tile/engine assignments
above are chosen so no instruction needs more than 2 semaphore lanes.
"""

import os
from contextlib import ExitStack

import numpy as np

import concourse.bass as bass
import concourse.mybir as mybir
import concourse.tile as tile
from concourse import masks
from concourse.bass_utils import run_bass_kernel_spmd

# Problem shapes (hardcoded per contract)
B, A, D, FA, FB, FP = 2048, 256, 8, 64, 16, 256
NCORES = 8
P = 128
G = 4                      # sub-tiles per super-tile
KC = 82                    # lhsT rows: 64 atoms + 16 bond sums + 1 bias + 1 pad
SPG = 16                   # samples accumulated per psum_out group

f32 = mybir.dt.float32
f32r = mybir.dt.float32r
f16 = mybir.dt.float16
i32 = mybir.dt.int32

# Set by kernel() after a run; test.py reads exec_time_ns / trace info.
LAST_RESULTS = None

RED_DTYPE = f16            # relu/mask dtype for the reduction matmul


def legalize_waits(nc, max_inline=1):
    """This toolchain's walrus accepts at most one semaphore wait inline per
    instruction (64B Events struct). Tile emits multi-wait sync_info; split
    the surplus into standalone EventSemaphore instructions just before the
    instruction on the same engine queue — identical semantics."""
    f = nc.m.functions[0]
    for bb in f.blocks:
        new = []
        for inst in bb.instructions:
            si = inst.sync_info
            waits = list(si.on_wait) if (si and si.on_wait) else []
            if len(waits) > max_inline:
                keep = waits[-max_inline:]
                moved = waits[:-max_inline]
                for k, w in enumerate(moved):
                    new.append(
                        mybir.InstEventSemaphore(
                            name=f"{inst.name}-prewait{k}",
                            ins=[],
                            outs=[],
                            sync_info=mybir.SyncInfo(on_wait=[w], on_update=[]),
                            engine=inst.engine,
                        )
                    )
                si.on_wait = keep
            new.append(inst)
        bb.instructions[:] = new


def build_nc(n_samples_per_core: int, legalize: bool = True) -> bass.Bass:
    """Build the single-core Bass program (same program runs SPMD on all cores)."""
    BC = n_samples_per_core
    N = BC * A                      # flat rows per core
    SUP = N // (G * P)              # super-tiles
    NT = N // P                     # sub-tiles
    SUPERS_PER_GROUP = SPG * A // (G * P)   # 8
    assert SUP % SUPERS_PER_GROUP == 0

    nc = bass.Bass()
    atoms_d = nc.dram_tensor("atoms", [N, FA], f32, kind="ExternalInput")
    bonds_d = nc.dram_tensor("bonds", [N, D * FB], f32, kind="ExternalInput")
    edges_d = nc.dram_tensor("edges", [N, D], i32, kind="ExternalInput")
    # host passes W stacked: rows 0:80 = W, row 80 = b, row 81 = 0 (pad)
    w_d = nc.dram_tensor("w", [KC, FP], f32, kind="ExternalInput")
    out_d = nc.dram_tensor("out", [BC, FP], f32, kind="ExternalOutput")

    with ExitStack() as ctx:
        tc = ctx.enter_context(tile.TileContext(nc))
        singles = ctx.enter_context(tc.tile_pool(name="singles", bufs=1))

        # ---- constants ----
        w_stage = singles.tile([KC, FP], f32)
        nc.sync.dma_start(out=w_stage[:], in_=w_d[:, :])
        # fp32r matmul operands must be rounded to fp32r by a producer op
        w_aug = singles.tile([KC, FP], f32r)
        nc.vector.tensor_scalar(
            out=w_aug[:], in0=w_stage[:], scalar1=0.0, scalar2=None,
            op0=mybir.AluOpType.add,
        )
        # identity built on gpsimd, then laundered through DVE so consumers
        # depend on a single engine lane
        identity_src = singles.tile([P, P], f32)
        masks.make_identity(nc, identity_src[:])
        identity = singles.tile([P, P], f32)
        nc.vector.tensor_copy(identity[:], identity_src[:])

        # mask16[:, t, s] = mask of row t*128+p if sub-tile t belongs to
        # sample slot s of its 16-sample group, else 0.
        mask16 = singles.tile([P, NT, SPG], RED_DTYPE)
        nc.vector.memset(mask16[:], 0.0)

        # cat buffers: manual rotation; bias col 80 = 1.0 and pad col 81 = 0.0
        # are preset once per buffer and never rewritten
        NB = 3
        cat_bufs = []
        for i in range(NB):
            cb = singles.tile([P, G, KC], f32, name=f"cat{i}")
            nc.vector.memset(cb[:, :, FA + FB : FA + FB + 1], 1.0)
            nc.vector.memset(cb[:, :, FA + FB + 1 : KC], 0.0)
            cat_bufs.append(cb)

        # PSUM pool for transposes — shared (same tag) between the prepass
        # and the main loop so slot reuse is PE-internal (no released-zone
        # cross-engine waits on the first main-loop transposes)
        psct = ctx.enter_context(tc.tile_pool(name="psct", bufs=2, space="PSUM"))

        # ---- mask pre-pass (pool stays alive: avoids release-zone deps) ----
        RPP = N // P  # rows per partition
        pp = ctx.enter_context(tc.tile_pool(name="prepass", bufs=1))
        if True:
            edges_sb = pp.tile([P, RPP * D], i32)
            nc.sync.dma_start(
                out=edges_sb[:],
                in_=edges_d[:, :].rearrange("(p r) d -> p (r d)", p=P),
            )
            degmax = pp.tile([P, RPP], i32)
            nc.vector.tensor_reduce(
                out=degmax[:],
                in_=edges_sb.rearrange("p (r d) -> p r d", d=D),
                axis=mybir.AxisListType.X,
                op=mybir.AluOpType.max,
            )
            # mask = (max_d edge >= 0) as 1.0/0.0
            masknat = pp.tile([P, RPP], f32)
            nc.vector.tensor_scalar(
                out=masknat[:], in0=degmax[:], scalar1=0, scalar2=None,
                op0=mybir.AluOpType.is_ge,
            )
            # maskT[:, t] = masks of rows [t*128, t*128+128)
            maskT = pp.tile([P, NT], f32)
            nblk = RPP // P  # 4 column-blocks
            maskT_v = maskT.rearrange("p (c j) -> p c j", j=nblk)
            for j in range(nblk):
                pst = psct.tile([P, P], f32, name="pst", tag="psum_ct")
                nc.tensor.transpose(
                    pst[:], masknat[:, j * P : (j + 1) * P], identity[:]
                )
                nc.vector.tensor_copy(maskT_v[:, :, j], pst[:])
            # scatter maskT columns into one-hot-by-sample-slot layout:
            # sub-tile t = 32u + 2s + h -> mask16 flat col 512u + 33s + 16h
            m16flat = mask16.rearrange("p t s -> p (t s)")
            maskT_w = maskT.rearrange("p (u w) -> p u w", w=2 * SPG)
            for s in range(SPG):
                for h in range(2):
                    dst = m16flat[:, 33 * s + SPG * h :: P * G]
                    nc.vector.tensor_copy(dst, maskT_w[:, :, 2 * s + h])

        # ---- main loop ----
        # atp/bondp bufs=4: with exactly 2 HWDGE DMAs per super over 8
        # round-robin lanes, T-4 is both the slot-WAW partner and the
        # lane-reuse predecessor -> one merged DMAHW wait (2-wait DMA limit)
        atp = ctx.enter_context(tc.tile_pool(name="atp", bufs=4))
        bondp = ctx.enter_context(tc.tile_pool(name="bondp", bufs=4))
        bs2p = ctx.enter_context(tc.tile_pool(name="bs2p", bufs=3))
        catTp = ctx.enter_context(tc.tile_pool(name="catTp", bufs=3))
        relup = ctx.enter_context(tc.tile_pool(name="relup", bufs=3))
        psfp = ctx.enter_context(tc.tile_pool(name="psfp", bufs=2, space="PSUM"))
        psout = ctx.enter_context(tc.tile_pool(name="psout", bufs=2, space="PSUM"))
        stagep = ctx.enter_context(tc.tile_pool(name="stagep", bufs=2))

        atoms_r = atoms_d[:, :].rearrange("(T g p) f -> T p g f", g=G, p=P)
        bonds_r = bonds_d[:, :].rearrange("(T g p) f -> T p g f", g=G, p=P)

        psum_out = None
        for T in range(SUP):
            cat = cat_bufs[T % NB]
            atile = atp.tile([P, G, FA], f32)
            nc.sync.dma_start(out=atile[:], in_=atoms_r[T])
            bonds_sb = bondp.tile([P, G, D * FB], f32)
            nc.sync.dma_start(out=bonds_sb[:], in_=bonds_r[T])

            # bond sum over D: gpsimd folds 8->4 into bondsum2, DVE reduces
            # 4->1 straight into cat cols [64:80]
            bview = bonds_sb.rearrange("p g (e x) -> p g e x", e=2)
            bondsum2 = bs2p.tile([P, G, (D // 2) * FB], f32)
            nc.gpsimd.tensor_tensor(
                out=bondsum2[:], in0=bview[:, :, 0], in1=bview[:, :, 1],
                op=mybir.AluOpType.add,
            )
            # atoms land in cat via DVE (tensor_scalar: 2-wait struct, and it
            # makes every cat writer a DVE op so the PE transposes — whose
            # weight-load allows a single semaphore wait — see one lane only).
            # Emission order (copy before reduce) lets the reduce's cat-WAR
            # PE wait elide against the copy's.
            nc.vector.tensor_scalar(
                out=cat[:, :, 0:FA], in0=atile[:], scalar1=0.0, scalar2=None,
                op0=mybir.AluOpType.add,
            )
            nc.vector.tensor_reduce(
                out=cat[:, :, FA : FA + FB],
                in_=bondsum2.rearrange("p g (d j) -> p g j d", d=D // 2),
                axis=mybir.AxisListType.X,
                op=mybir.AluOpType.add,
            )

            # feature-major transpose via PE (fp32; fp32r rounding happens
            # in the PSUM->SBUF drain copy)
            psum_ct = psct.tile([KC, G * P], f32, tag="psum_ct")
            for g in range(G):
                nc.tensor.transpose(
                    psum_ct[:, g * P : (g + 1) * P],
                    cat[:, g, :],
                    identity[:],
                )
            catT = catTp.tile([KC, G * P], f32r)
            nc.vector.tensor_scalar(
                out=catT[:], in0=psum_ct[:, :],
                scalar1=0.0, scalar2=None, op0=mybir.AluOpType.add,
            )

            # main matmul (bias folded in via ones row 96)
            psum_fp = psfp.tile([P, G * FP], f32)
            for g in range(G):
                nc.tensor.matmul(
                    psum_fp[:, g * FP : (g + 1) * FP],
                    lhsT=catT[:, g * P : (g + 1) * P],
                    rhs=w_aug[:, :],
                    start=True,
                    stop=True,
                )

            relu = relup.tile([P, G * FP], RED_DTYPE)
            nc.scalar.activation(
                relu[:], psum_fp[:], mybir.ActivationFunctionType.Relu
            )

            # mask-weighted atom reduction
            if T % SUPERS_PER_GROUP == 0:
                psum_out = psout.tile([SPG, FP], f32, name="psum_out")
            for g in range(G):
                t = G * T + g
                nc.tensor.matmul(
                    psum_out[:, :],
                    lhsT=mask16[:, t, :],
                    rhs=relu[:, g * FP : (g + 1) * FP],
                    start=(t % (2 * SPG) == 0),
                    stop=(t % (2 * SPG) == 2 * SPG - 1),
                )
            if T % SUPERS_PER_GROUP == SUPERS_PER_GROUP - 1:
                grp = T // SUPERS_PER_GROUP
                stage = stagep.tile([SPG, FP], f32)
                nc.scalar.copy(stage[:], psum_out[:])
                # SWDGE so the output drain doesn't perturb the HWDGE
                # lane rotation that atp/bondp bufs=4 relies on
                nc.gpsimd.dma_start(
                    out=out_d[grp * SPG : (grp + 1) * SPG, :], in_=stage[:]
                )
    if legalize:
        legalize_waits(nc)
    return nc


def stack_w(W, b):
    """Host-side W layout matching catT rows: W | bias | zero pad."""
    return np.ascontiguousarray(
        np.vstack(
            [
                np.asarray(W, dtype=np.float32),
                np.asarray(b, dtype=np.float32).reshape(1, FP),
                np.zeros((KC - 2 - FA - FB + 1, FP), dtype=np.float32),
            ]
        )
    )


def _shard_inputs(atoms, bonds, edges, W, b, n_samples_per_core):
    BC = n_samples_per_core
    N = BC * A
    in_maps = []
    w_np = stack_w(W, b)
    for c in range(NCORES):
        sl = slice(c * BC, (c + 1) * BC)
        in_maps.append(
            {
                "atoms": np.ascontiguousarray(
                    np.asarray(atoms[sl], dtype=np.float32).reshape(N, FA)
                ),
                "bonds": np.ascontiguousarray(
                    np.asarray(bonds[sl], dtype=np.float32).reshape(N, D * FB)
                ),
                "edges": np.ascontiguousarray(
                    np.asarray(edges[sl], dtype=np.int32).reshape(N, D)
                ),
                "w": w_np,
            }
        )
    return in_maps


def postprocess(concat_out):
    """Map concatenated per-core 'out' buffers to the full [B, FP] output."""
    return np.ascontiguousarray(np.asarray(concat_out).reshape(B, FP))


def kernel(atoms, bonds, edges, W, b):
    """Full inputs in, full output out. Shards batch across 8 cores."""
    global LAST_RESULTS
    BC = B // NCORES
    nc = build_nc(BC)
    in_maps = _shard_inputs(atoms, bonds, edges, W, b, BC)
    core_ids = list(range(NCORES))
    trace = bool(os.environ.get("KERNEL_TRACE"))
    res = run_bass_kernel_spmd(nc, in_maps, core_ids, trace=trace)
    LAST_RESULTS = res
    out = np.concatenate([res.results[c]["out"] for c in range(NCORES)], axis=0)
    return out.astype(np.float32)

